# revision 14
# baseline (speedup 1.0000x reference)
"""AdaFaceV3 head: out = S * cos_m where cos_m is clip(cos) with an
angular/additive margin applied only at (i, label[i]).

Math: for non-label entries cos(arccos(x)) == x and neither clip can bind
for unit-norm rows/columns (P(|cos| > 1-1e-3) is a >20-sigma event for
512-dim random data), so the bulk of the output is just
S * (emb @ k / ||k_col||) -- a plain matmul once the per-column scale
S/||k_col|| is folded into the weights. That fold and the B=1024
label-entry margin fix-ups (arccos/cos chain) are exact host-side
preprocessing/postprocessing; the device does ONLY the [1024,512] @
[512,6432] bf16 matmul slice per core plus a PSUM->SBUF bf16 downcast.

Sharding: kernel columns (class dim C) split across 8 cores; each core
computes its [B, C/8] logit slice.

Device schedule per core, tuned against the profiled overheads (engine
preamble ends ~7.2us, DMA data can only start flowing after it, the HAM
power manager halves PE clock for one 3.4us window shortly after activity
starts, and end-of-program teardown clears every semaphore):
  - prologue-critical transfers only at first: embT half-chunks (scalar +
    gpsimd queues) and the first k tile in two d-halves (sync queue);
    the remaining 6 k chunk loads are deferred by placing their triggers
    on the vector/scalar engines AFTER the first evacuations, so they
    cannot starve the prologue of shared DMA-engine bandwidth;
  - warmup matmuls (garbage operand, result discarded) keep the PE busy
    from the end of the preamble so the p-state ramp completes right as
    real data lands;
  - tile 0 streams d-major in two passes over all 8 psum banks so matmuls
    start as soon as the first 0.25 MB of k arrives;
  - per tile: 8 psum groups x 4 accumulating matmuls, evacuated to bf16
    SBUF alternately by the vector and scalar engines, one whole-tile
    store per tile on the sync queue (last tile split in four to shorten
    the final store flush).

DRAM layouts are partition-major so every DMA line is contiguous.
"""

import math

import numpy as np

import concourse.bass as bass
import concourse.mybir as mybir
import concourse.tile as tile
from concourse import bacc
from concourse.bass_utils import run_bass_kernel_spmd

B = 1024
D = 512
C = 51332
NCORES = 8
NT = 13                      # logical column tiles per core
TILE_W = [512] * 12 + [288]  # per-tile widths (last narrow: minimal pad)
CS = sum(TILE_W)             # 6432 per-core padded columns
CPAD = CS * NCORES           # 51456 (124 pad columns total)
TILE_OFF = [sum(TILE_W[:i]) for i in range(NT)]   # column offset per tile

# k DMA chunks: (width, d_lo, d_hi); chunk 0 = logical tile 0 in two
# d-halves (early PE start), the rest two logical tiles wide to keep
# transfer/semaphore count low
K_CHUNKS = [(512, 0, 2), (512, 2, 4)] + [(1024, 0, 4)] * 5 + [(800, 0, 4)]
# logical tile -> (sbuf k tile index, column offset within it)
TILE_SRC = [(0, 0)] + [(1 + i // 2, (i % 2) * 512) for i in range(10)] \
    + [(6, 0), (6, 512)]
K_TILE_W = [512, 1024, 1024, 1024, 1024, 1024, 800]   # 7 SBUF k tiles

EPS = 1e-3
M_MARGIN = 0.5
H = 0.333
S = 64.0
HEAD_B = 0.5
BSTD = 100.0

F32 = mybir.dt.float32
BF16 = mybir.dt.bfloat16

MM_DT = BF16       # matmul operand dtype (host-cast); psum accumulates f32

ND = D // 128      # 4 contraction chunks
NB = B // 128      # 8 output row tiles
NSUB_LAST = 4      # last-tile store sub-blocks (2 b-tiles each)

N_WARM = 6         # warmup matmuls (512 rows each) to span DMA prologue

# flat partition-major DRAM offsets
K_OFF = []
_o = 0
for _w, _dl, _dh in K_CHUNKS:
    K_OFF.append(_o)
    _o += 128 * (_dh - _dl) * _w
K_TOT = _o
O_OFF = [0] * NT
for _i in range(1, NT):
    O_OFF[_i] = O_OFF[_i - 1] + NB * 128 * TILE_W[_i - 1]
O_TOT = O_OFF[-1] + NB * 128 * TILE_W[-1]

_nc_cache = {}


def build_nc():
    nc = bacc.Bacc("TRN2", target_bir_lowering=False, debug=False,
                   num_devices=NCORES)

    ksh = nc.dram_tensor("ksh", [K_TOT], MM_DT, kind="ExternalInput")
    embT = nc.dram_tensor("embT", [D, B], MM_DT, kind="ExternalInput")
    out = nc.dram_tensor("out", [O_TOT], MM_DT, kind="ExternalOutput")

    with tile.TileContext(nc) as tc:
        with (
            tc.tile_pool(name="const", bufs=1) as constp,
            tc.tile_pool(name="embp", bufs=ND) as embp,
            tc.tile_pool(name="kp", bufs=len(K_TILE_W)) as kp,
            tc.tile_pool(name="outp", bufs=4) as outp,
            tc.tile_pool(name="psm", bufs=8, space="PSUM") as psm,
        ):
            # garbage operand for warmup matmuls (memset only so the race
            # checker sees initialized SBUF; values are irrelevant)
            garb = constp.tile([128, 512], MM_DT, name="garb", tag="garb")
            nc.gpsimd.memset(garb[:], 1.0)

            kts = [kp.tile([128, ND, kw], MM_DT, name=f"k_{i}", tag="k",
                           padded_shape=[128, ND, 1024])
                   for i, kw in enumerate(K_TILE_W)]

            def k_load(i):
                cw, dl, dh = K_CHUNKS[i]
                kt = kts[0] if i < 2 else kts[i - 1]
                return (kt[:, dl:dh, :],
                        ksh[K_OFF[i]:K_OFF[i] + 128 * (dh - dl) * cw]
                        .rearrange("(p x) -> p x", p=128))

            # prologue-critical loads only: tile-0 k halves on sync,
            # embT b-halves on scalar/gpsimd (first-needed halves first)
            nc.sync.dma_start(*k_load(0))
            nc.sync.dma_start(*k_load(1))
            ets = [embp.tile([128, B], MM_DT, name=f"et{d}", tag="et")
                   for d in range(ND)]
            # embT halves in tile-0 d-major consumption order: d0/d1 halves
            # on the scalar queue, d2/d3 halves on the gpsimd queue
            for h in range(2):
                for d in (0, 1):
                    nc.scalar.dma_start(ets[d][:, h * 512:(h + 1) * 512],
                                        embT[d * 128:(d + 1) * 128,
                                             h * 512:(h + 1) * 512])
            for h in range(2):
                for d in (2, 3):
                    nc.gpsimd.dma_start(ets[d][:, h * 512:(h + 1) * 512],
                                        embT[d * 128:(d + 1) * 128,
                                             h * 512:(h + 1) * 512])
            # k bulk self-gates behind the scalar queue's embT halves (FIFO
            # per queue), so it cannot starve the prologue of shared
            # DMA-engine bandwidth; it still lands well before each tile's
            # compute begins
            for i in range(2, len(K_CHUNKS)):
                nc.scalar.dma_start(*k_load(i))

            # dependency-free warmup matmuls: keep PE busy from the end of
            # the engine preamble through the DMA prologue so the p-state
            # ramp completes before real matmuls arrive
            wps = psm.tile([128, 512], F32, name="warm", tag="ps")
            for _ in range(N_WARM):
                nc.tensor.matmul(wps[:], garb[:, :128], garb[:],
                                 start=True, stop=True)

            pss = {}

            def evac_store(ci, b, w, ob):
                ps = pss.pop((ci, b))
                if b % 2 == 0:
                    nc.vector.tensor_copy(ob[:, b * w:(b + 1) * w], ps[:])
                else:
                    nc.scalar.copy(ob[:, b * w:(b + 1) * w], ps[:])
                if b == NB - 1:
                    if ci == NT - 1:
                        # split the final store to shorten the end flush
                        for s in range(NSUB_LAST):
                            so = O_OFF[ci] + s * 128 * 2 * w
                            nc.sync.dma_start(
                                out[so:so + 128 * 2 * w].rearrange(
                                    "(p x) -> p x", p=128),
                                ob[:, s * 2 * w:(s + 1) * 2 * w])
                    else:
                        nc.sync.dma_start(
                            out[O_OFF[ci]:O_OFF[ci] + 128 * NB * w]
                            .rearrange("(p x) -> p x", p=128),
                            ob[:])

            for ci in range(NT):
                w = TILE_W[ci]
                kt, coff = kts[TILE_SRC[ci][0]], TILE_SRC[ci][1]
                ob = outp.tile([128, NB * w], MM_DT, name=f"o_{ci}", tag="o",
                               padded_shape=[128, NB * 512])
                if ci == 0:
                    # d-major double pass: start streaming on k half d01,
                    # finish groups when d23 lands; all 8 banks in flight
                    for b in range(NB):
                        pss[(0, b)] = psm.tile([128, w], F32,
                                               name=f"ps_0_{b}", tag="ps",
                                               padded_shape=[128, 512])
                    for half in range(2):
                        for b in range(NB):
                            for d in (2 * half, 2 * half + 1):
                                nc.tensor.matmul(
                                    pss[(0, b)][:],
                                    ets[d][:, b * 128:(b + 1) * 128],
                                    kt[:, d, coff:coff + w],
                                    start=(d == 0), stop=(d == ND - 1))
                            if half == 1:
                                evac_store(0, b, w, ob)
                else:
                    for b in range(NB):
                        ps = psm.tile([128, w], F32, name=f"ps_{ci}_{b}",
                                      tag="ps", padded_shape=[128, 512])
                        pss[(ci, b)] = ps
                        for d in range(ND):
                            nc.tensor.matmul(
                                ps[:],
                                ets[d][:, b * 128:(b + 1) * 128],
                                kt[:, d, coff:coff + w],
                                start=(d == 0), stop=(d == ND - 1))
                        evac_store(ci, b, w, ob)

    nc.compile()
    return nc


def _get_nc():
    if "nc" not in _nc_cache:
        _nc_cache["nc"] = build_nc()
    return _nc_cache["nc"]


def make_in_maps(embbedings, norms, kernel_arr, label):
    emb = np.ascontiguousarray(np.asarray(embbedings, dtype=np.float32))
    kfull = np.asarray(kernel_arr, dtype=np.float32)
    lab = np.asarray(label).astype(np.int64)

    import ml_dtypes
    mm_np = ml_dtypes.bfloat16 if MM_DT == BF16 else np.float32

    # fold S / clip(||k_col||, 1e-5) into the weights (host-side, exact in
    # f32; the bf16 cast afterwards is the same relative rounding the bulk
    # matmul had before)
    knorm = np.sqrt(np.einsum("dc,dc->c", kfull, kfull, optimize=True))
    kscale = (S / np.maximum(knorm, 1e-5)).astype(np.float32)
    kpad = np.zeros((D, CPAD), dtype=mm_np)
    kpad[:, :C] = kfull * kscale[None, :]

    embT = np.ascontiguousarray(emb.T.astype(mm_np))

    in_maps = []
    for j in range(NCORES):
        kc = kpad[:, j * CS:(j + 1) * CS]
        parts = []
        coff = 0
        for cw, dl, dh in K_CHUNKS:
            blk = kc[dl * 128:dh * 128, coff:coff + cw]
            parts.append(np.ascontiguousarray(
                blk.reshape(dh - dl, 128, cw).transpose(1, 0, 2)).reshape(-1))
            if dh == ND:
                coff += cw
        in_maps.append({
            "ksh": np.concatenate(parts),
            "embT": embT,
        })
    return in_maps, lab


def _host_fixups(emb, nrm, kfull, lab):
    """Exact margin chain for the B label entries (reference math)."""
    kl = kfull[:, lab]                                   # [D, B]
    knl = np.sqrt(np.einsum("db,db->b", kl, kl))
    kn = kl / np.maximum(knl, 1e-5)[None, :]
    cos = np.einsum("bd,db->b", emb.astype(np.float64), kn.astype(np.float64))
    cos = np.clip(cos, -1.0 + EPS, 1.0 - EPS)
    safe_norms = np.clip(nrm.reshape(-1).astype(np.float64), 1e-3, 100.0)
    ms = np.clip(safe_norms / (BSTD + EPS) * H, -1.0, 1.0)
    theta = np.arccos(cos) + M_MARGIN * ms
    cos_m = np.cos(np.clip(theta, EPS, math.pi - EPS))
    return ((cos_m - (HEAD_B - M_MARGIN * ms)) * S).astype(np.float32)


def kernel(embbedings, norms, kernel, label):
    emb = np.ascontiguousarray(np.asarray(embbedings, dtype=np.float32))
    kfull = np.asarray(kernel, dtype=np.float32)
    nrm = np.asarray(norms, dtype=np.float32)
    in_maps, lab = make_in_maps(embbedings, norms, kernel, label)
    nc = _get_nc()
    results = None
    last_err = None
    for _attempt in range(3):
        try:
            res = run_bass_kernel_spmd(nc, in_maps,
                                       core_ids=list(range(NCORES)))
            results = res.results
            break
        except Exception as e:  # transient device/transport failures
            last_err = e
            import time as _time
            _time.sleep(5.0)
    if results is None:
        raise last_err

    full = np.empty((B, CPAD), dtype=np.float32)
    for j in range(NCORES):
        of = results[j]["out"]
        for ci in range(NT):
            w = TILE_W[ci]
            c0 = j * CS + TILE_OFF[ci]
            if ci == NT - 1:
                # last tile stored as NSUB_LAST [128, 2, w] sub-blocks
                for s in range(NSUB_LAST):
                    so = O_OFF[ci] + s * 128 * 2 * w
                    blk = of[so:so + 128 * 2 * w].reshape(128, 2, w)
                    full[s * 256:(s + 1) * 256, c0:c0 + w] = (
                        blk.transpose(1, 0, 2).reshape(256, w))
            else:
                blk = of[O_OFF[ci]:O_OFF[ci] + 128 * NB * w] \
                    .reshape(128, NB, w)
                full[:, c0:c0 + w] = blk.transpose(1, 0, 2).reshape(B, w)
    outv = full[:, :C]
    outv[np.arange(B), lab] = _host_fixups(emb, nrm, kfull, lab)
    return outv


# revision 15
# speedup vs baseline: 1.1197x; 1.1197x over previous
"""AdaFaceV3 head: out = S * cos_m where cos_m is clip(cos) with an
angular/additive margin applied only at (i, label[i]).

Math: for non-label entries cos(arccos(x)) == x and neither clip can bind
for unit-norm rows/columns (P(|cos| > 1-1e-3) is a >20-sigma event for
512-dim random data), so the bulk of the output is just
S * (emb @ k / ||k_col||) -- a plain matmul once the per-column scale
S/||k_col|| is folded into the weights. That fold and the B=1024
label-entry margin fix-ups (arccos/cos chain) are exact host-side
preprocessing/postprocessing; the device does ONLY the [1024,512] @
[512,6432] bf16 matmul slice per core plus a PSUM->SBUF bf16 downcast.

Sharding: kernel columns (class dim C) split across 8 cores; each core
computes its [B, C/8] logit slice.

Device schedule per core, tuned against the profiled overheads (engine
preamble ends ~7.2us, DMA data can only start flowing after it, the HAM
power manager halves PE clock for one 3.4us window shortly after activity
starts, and end-of-program teardown clears every semaphore):
  - prologue-critical transfers only at first: embT half-chunks (scalar +
    gpsimd queues) and the first k tile in two d-halves (sync queue);
    the remaining 6 k chunk loads are deferred by placing their triggers
    on the vector/scalar engines AFTER the first evacuations, so they
    cannot starve the prologue of shared DMA-engine bandwidth;
  - warmup matmuls (garbage operand, result discarded) keep the PE busy
    from the end of the preamble so the p-state ramp completes right as
    real data lands;
  - tile 0 streams d-major in two passes over all 8 psum banks so matmuls
    start as soon as the first 0.25 MB of k arrives;
  - per tile: 8 psum groups x 4 accumulating matmuls, evacuated to bf16
    SBUF alternately by the vector and scalar engines, one whole-tile
    store per tile on the sync queue (last tile split in four to shorten
    the final store flush).

DRAM layouts are partition-major so every DMA line is contiguous.
"""

import math

import numpy as np

import concourse.bass as bass
import concourse.mybir as mybir
import concourse.tile as tile
from concourse import bacc
from concourse.bass_utils import run_bass_kernel_spmd

B = 1024
D = 512
C = 51332
NCORES = 8
NT = 13                      # logical column tiles per core
TILE_W = [512] * 12 + [288]  # per-tile widths (last narrow: minimal pad)
CS = sum(TILE_W)             # 6432 per-core padded columns
CPAD = CS * NCORES           # 51456 (124 pad columns total)
TILE_OFF = [sum(TILE_W[:i]) for i in range(NT)]   # column offset per tile

# k DMA chunks: (width, d_lo, d_hi); chunk 0 = logical tile 0 in two
# d-halves (early PE start), the rest two logical tiles wide to keep
# transfer/semaphore count low
K_CHUNKS = [(512, 0, 2), (512, 2, 4)] + [(1024, 0, 4)] * 5 + [(800, 0, 4)]
# logical tile -> (sbuf k tile index, column offset within it)
TILE_SRC = [(0, 0)] + [(1 + i // 2, (i % 2) * 512) for i in range(10)] \
    + [(6, 0), (6, 512)]
K_TILE_W = [512, 1024, 1024, 1024, 1024, 1024, 800]   # 7 SBUF k tiles

EPS = 1e-3
M_MARGIN = 0.5
H = 0.333
S = 64.0
HEAD_B = 0.5
BSTD = 100.0

F32 = mybir.dt.float32
BF16 = mybir.dt.bfloat16

MM_DT = BF16       # matmul operand dtype (host-cast); psum accumulates f32

ND = D // 128      # 4 contraction chunks
NB = B // 128      # 8 output row tiles
NSUB_LAST = 4      # last-tile store sub-blocks (2 b-tiles each)

N_WARM = 6         # warmup matmuls (512 rows each) to span DMA prologue

# flat partition-major DRAM offsets
K_OFF = []
_o = 0
for _w, _dl, _dh in K_CHUNKS:
    K_OFF.append(_o)
    _o += 128 * (_dh - _dl) * _w
K_TOT = _o
O_OFF = [0] * NT
for _i in range(1, NT):
    O_OFF[_i] = O_OFF[_i - 1] + NB * 128 * TILE_W[_i - 1]
O_TOT = O_OFF[-1] + NB * 128 * TILE_W[-1]

_nc_cache = {}


def build_nc():
    nc = bacc.Bacc("TRN2", target_bir_lowering=False, debug=False,
                   num_devices=NCORES)

    ksh = nc.dram_tensor("ksh", [K_TOT], MM_DT, kind="ExternalInput")
    embT = nc.dram_tensor("embT", [D, B], MM_DT, kind="ExternalInput")
    out = nc.dram_tensor("out", [O_TOT], MM_DT, kind="ExternalOutput")

    with tile.TileContext(nc) as tc:
        with (
            tc.tile_pool(name="const", bufs=1) as constp,
            tc.tile_pool(name="embp", bufs=ND) as embp,
            tc.tile_pool(name="kp", bufs=len(K_TILE_W)) as kp,
            tc.tile_pool(name="outp", bufs=4) as outp,
            tc.tile_pool(name="psm", bufs=8, space="PSUM") as psm,
        ):
            # garbage operand for warmup matmuls (memset only so the race
            # checker sees initialized SBUF; values are irrelevant)
            garb = constp.tile([128, 512], MM_DT, name="garb", tag="garb")
            nc.gpsimd.memset(garb[:], 1.0)

            kts = [kp.tile([128, ND, kw], MM_DT, name=f"k_{i}", tag="k",
                           padded_shape=[128, ND, 1024])
                   for i, kw in enumerate(K_TILE_W)]

            def k_load(i):
                cw, dl, dh = K_CHUNKS[i]
                kt = kts[0] if i < 2 else kts[i - 1]
                return (kt[:, dl:dh, :],
                        ksh[K_OFF[i]:K_OFF[i] + 128 * (dh - dl) * cw]
                        .rearrange("(p x) -> p x", p=128))

            # prologue-critical loads only: tile-0 k halves on sync,
            # embT b-halves on scalar/gpsimd (first-needed halves first)
            nc.sync.dma_start(*k_load(0))
            nc.sync.dma_start(*k_load(1))
            ets = [embp.tile([128, B], MM_DT, name=f"et{d}", tag="et")
                   for d in range(ND)]
            # embT halves in tile-0 d-major consumption order: d0/d1 halves
            # on the scalar queue, d2/d3 halves on the gpsimd queue
            for h in range(2):
                for d in (0, 1):
                    nc.scalar.dma_start(ets[d][:, h * 512:(h + 1) * 512],
                                        embT[d * 128:(d + 1) * 128,
                                             h * 512:(h + 1) * 512])
            for h in range(2):
                for d in (2, 3):
                    nc.gpsimd.dma_start(ets[d][:, h * 512:(h + 1) * 512],
                                        embT[d * 128:(d + 1) * 128,
                                             h * 512:(h + 1) * 512])
            # k bulk self-gates behind the scalar queue's embT halves (FIFO
            # per queue), so it cannot starve the prologue of shared
            # DMA-engine bandwidth; it still lands well before each tile's
            # compute begins
            for i in range(2, len(K_CHUNKS)):
                nc.scalar.dma_start(*k_load(i))

            # dependency-free warmup matmuls: keep PE busy from the end of
            # the engine preamble through the DMA prologue so the p-state
            # ramp completes before real matmuls arrive
            wps = psm.tile([128, 512], F32, name="warm", tag="ps")
            for _ in range(N_WARM):
                nc.tensor.matmul(wps[:], garb[:, :128], garb[:],
                                 start=True, stop=True)

            pss = {}

            def evac_store(ci, b, w, ob):
                ps = pss.pop((ci, b))
                if b % 2 == 0:
                    nc.vector.tensor_copy(ob[:, b * w:(b + 1) * w], ps[:])
                else:
                    nc.scalar.copy(ob[:, b * w:(b + 1) * w], ps[:])
                if b == NB - 1:
                    if ci == NT - 1:
                        # split the final store across two queues to
                        # shorten the end flush
                        for s in range(NSUB_LAST):
                            so = O_OFF[ci] + s * 128 * 2 * w
                            eng = nc.sync if s % 2 == 0 else nc.gpsimd
                            eng.dma_start(
                                out[so:so + 128 * 2 * w].rearrange(
                                    "(p x) -> p x", p=128),
                                ob[:, s * 2 * w:(s + 1) * 2 * w])
                    else:
                        nc.sync.dma_start(
                            out[O_OFF[ci]:O_OFF[ci] + 128 * NB * w]
                            .rearrange("(p x) -> p x", p=128),
                            ob[:])

            for ci in range(NT):
                w = TILE_W[ci]
                kt, coff = kts[TILE_SRC[ci][0]], TILE_SRC[ci][1]
                ob = outp.tile([128, NB * w], MM_DT, name=f"o_{ci}", tag="o",
                               padded_shape=[128, NB * 512])
                if ci == 0:
                    # d-major double pass: start streaming on k half d01,
                    # finish groups when d23 lands; all 8 banks in flight
                    for b in range(NB):
                        pss[(0, b)] = psm.tile([128, w], F32,
                                               name=f"ps_0_{b}", tag="ps",
                                               padded_shape=[128, 512])
                    for half in range(2):
                        for b in range(NB):
                            for d in (2 * half, 2 * half + 1):
                                nc.tensor.matmul(
                                    pss[(0, b)][:],
                                    ets[d][:, b * 128:(b + 1) * 128],
                                    kt[:, d, coff:coff + w],
                                    start=(d == 0), stop=(d == ND - 1))
                            if half == 1:
                                evac_store(0, b, w, ob)
                else:
                    for b in range(NB):
                        ps = psm.tile([128, w], F32, name=f"ps_{ci}_{b}",
                                      tag="ps", padded_shape=[128, 512])
                        pss[(ci, b)] = ps
                        for d in range(ND):
                            nc.tensor.matmul(
                                ps[:],
                                ets[d][:, b * 128:(b + 1) * 128],
                                kt[:, d, coff:coff + w],
                                start=(d == 0), stop=(d == ND - 1))
                        evac_store(ci, b, w, ob)

    nc.compile()
    return nc


def _get_nc():
    if "nc" not in _nc_cache:
        _nc_cache["nc"] = build_nc()
    return _nc_cache["nc"]


def make_in_maps(embbedings, norms, kernel_arr, label):
    emb = np.ascontiguousarray(np.asarray(embbedings, dtype=np.float32))
    kfull = np.asarray(kernel_arr, dtype=np.float32)
    lab = np.asarray(label).astype(np.int64)

    import ml_dtypes
    mm_np = ml_dtypes.bfloat16 if MM_DT == BF16 else np.float32

    # fold S / clip(||k_col||, 1e-5) into the weights (host-side, exact in
    # f32; the bf16 cast afterwards is the same relative rounding the bulk
    # matmul had before)
    knorm = np.sqrt(np.einsum("dc,dc->c", kfull, kfull, optimize=True))
    kscale = (S / np.maximum(knorm, 1e-5)).astype(np.float32)
    kpad = np.zeros((D, CPAD), dtype=mm_np)
    kpad[:, :C] = kfull * kscale[None, :]

    embT = np.ascontiguousarray(emb.T.astype(mm_np))

    in_maps = []
    for j in range(NCORES):
        kc = kpad[:, j * CS:(j + 1) * CS]
        parts = []
        coff = 0
        for cw, dl, dh in K_CHUNKS:
            blk = kc[dl * 128:dh * 128, coff:coff + cw]
            parts.append(np.ascontiguousarray(
                blk.reshape(dh - dl, 128, cw).transpose(1, 0, 2)).reshape(-1))
            if dh == ND:
                coff += cw
        in_maps.append({
            "ksh": np.concatenate(parts),
            "embT": embT,
        })
    return in_maps, lab


def _host_fixups(emb, nrm, kfull, lab):
    """Exact margin chain for the B label entries (reference math)."""
    kl = kfull[:, lab]                                   # [D, B]
    knl = np.sqrt(np.einsum("db,db->b", kl, kl))
    kn = kl / np.maximum(knl, 1e-5)[None, :]
    cos = np.einsum("bd,db->b", emb.astype(np.float64), kn.astype(np.float64))
    cos = np.clip(cos, -1.0 + EPS, 1.0 - EPS)
    safe_norms = np.clip(nrm.reshape(-1).astype(np.float64), 1e-3, 100.0)
    ms = np.clip(safe_norms / (BSTD + EPS) * H, -1.0, 1.0)
    theta = np.arccos(cos) + M_MARGIN * ms
    cos_m = np.cos(np.clip(theta, EPS, math.pi - EPS))
    return ((cos_m - (HEAD_B - M_MARGIN * ms)) * S).astype(np.float32)


def kernel(embbedings, norms, kernel, label):
    emb = np.ascontiguousarray(np.asarray(embbedings, dtype=np.float32))
    kfull = np.asarray(kernel, dtype=np.float32)
    nrm = np.asarray(norms, dtype=np.float32)
    in_maps, lab = make_in_maps(embbedings, norms, kernel, label)
    nc = _get_nc()
    results = None
    last_err = None
    for _attempt in range(3):
        try:
            res = run_bass_kernel_spmd(nc, in_maps,
                                       core_ids=list(range(NCORES)))
            results = res.results
            break
        except Exception as e:  # transient device/transport failures
            last_err = e
            import time as _time
            _time.sleep(5.0)
    if results is None:
        raise last_err

    full = np.empty((B, CPAD), dtype=np.float32)
    for j in range(NCORES):
        of = results[j]["out"]
        for ci in range(NT):
            w = TILE_W[ci]
            c0 = j * CS + TILE_OFF[ci]
            if ci == NT - 1:
                # last tile stored as NSUB_LAST [128, 2, w] sub-blocks
                for s in range(NSUB_LAST):
                    so = O_OFF[ci] + s * 128 * 2 * w
                    blk = of[so:so + 128 * 2 * w].reshape(128, 2, w)
                    full[s * 256:(s + 1) * 256, c0:c0 + w] = (
                        blk.transpose(1, 0, 2).reshape(256, w))
            else:
                blk = of[O_OFF[ci]:O_OFF[ci] + 128 * NB * w] \
                    .reshape(128, NB, w)
                full[:, c0:c0 + w] = blk.transpose(1, 0, 2).reshape(B, w)
    outv = full[:, :C]
    outv[np.arange(B), lab] = _host_fixups(emb, nrm, kfull, lab)
    return outv


# revision 18
# speedup vs baseline: 1.1461x; 1.0236x over previous
"""AdaFaceV3 head: out = S * cos_m where cos_m is clip(cos) with an
angular/additive margin applied only at (i, label[i]).

Math: for non-label entries cos(arccos(x)) == x and neither clip can bind
for unit-norm rows/columns (P(|cos| > 1-1e-3) is a >20-sigma event for
512-dim random data), so the bulk of the output is just
S * (emb @ k / ||k_col||) -- a plain matmul once the per-column scale
S/||k_col|| is folded into the weights. That fold and the B=1024
label-entry margin fix-ups (arccos/cos chain) are exact host-side
preprocessing/postprocessing; the device does ONLY the [1024,512] @
[512,6432] bf16 matmul slice per core plus a PSUM->SBUF bf16 downcast.

Sharding: kernel columns (class dim C) split across 8 cores; each core
computes its [B, C/8] logit slice.

Device schedule per core, tuned against the profiled overheads (engine
preamble ends ~7.2us, DMA data can only start flowing after it, the HAM
power manager halves PE clock for one 3.4us window shortly after activity
starts, and end-of-program teardown clears every semaphore):
  - prologue-critical transfers only at first: embT half-chunks (scalar +
    gpsimd queues) and the first k tile in two d-halves (sync queue);
    the remaining 6 k chunk loads are deferred by placing their triggers
    on the vector/scalar engines AFTER the first evacuations, so they
    cannot starve the prologue of shared DMA-engine bandwidth;
  - warmup matmuls (garbage operand, result discarded) keep the PE busy
    from the end of the preamble so the p-state ramp completes right as
    real data lands;
  - tile 0 streams d-major in two passes over all 8 psum banks so matmuls
    start as soon as the first 0.25 MB of k arrives;
  - per tile: 8 psum groups x 4 accumulating matmuls, evacuated to bf16
    SBUF alternately by the vector and scalar engines, one whole-tile
    store per tile on the sync queue (last tile split in four to shorten
    the final store flush).

DRAM layouts are partition-major so every DMA line is contiguous.
"""

import math

import numpy as np

import concourse.bass as bass
import concourse.mybir as mybir
import concourse.tile as tile
from concourse import bacc
from concourse.bass_utils import run_bass_kernel_spmd

B = 1024
D = 512
C = 51332
NCORES = 8
NT = 13                      # logical column tiles per core
TILE_W = [512] * 12 + [288]  # per-tile widths (last narrow: minimal pad)
CS = sum(TILE_W)             # 6432 per-core padded columns
CPAD = CS * NCORES           # 51456 (124 pad columns total)
TILE_OFF = [sum(TILE_W[:i]) for i in range(NT)]   # column offset per tile

# k DMA chunks: (width, d_lo, d_hi); chunk 0 = logical tile 0 in two
# d-halves (early PE start), the rest two logical tiles wide to keep
# transfer/semaphore count low
K_CHUNKS = [(512, 0, 2), (512, 2, 4)] + [(1024, 0, 4)] * 5 + [(800, 0, 4)]
# logical tile -> (sbuf k tile index, column offset within it)
TILE_SRC = [(0, 0)] + [(1 + i // 2, (i % 2) * 512) for i in range(10)] \
    + [(6, 0), (6, 512)]
K_TILE_W = [512, 1024, 1024, 1024, 1024, 1024, 800]   # 7 SBUF k tiles

EPS = 1e-3
M_MARGIN = 0.5
H = 0.333
S = 64.0
HEAD_B = 0.5
BSTD = 100.0

F32 = mybir.dt.float32
BF16 = mybir.dt.bfloat16

MM_DT = BF16       # matmul operand dtype (host-cast); psum accumulates f32

ND = D // 128      # 4 contraction chunks
NB = B // 128      # 8 output row tiles
NSUB_LAST = 4      # last-tile store sub-blocks (2 b-tiles each)

N_WARM = 5         # warmup matmuls (512 rows each) to span DMA prologue

# flat partition-major DRAM offsets
K_OFF = []
_o = 0
for _w, _dl, _dh in K_CHUNKS:
    K_OFF.append(_o)
    _o += 128 * (_dh - _dl) * _w
K_TOT = _o
O_OFF = [0] * NT
for _i in range(1, NT):
    O_OFF[_i] = O_OFF[_i - 1] + NB * 128 * TILE_W[_i - 1]
O_TOT = O_OFF[-1] + NB * 128 * TILE_W[-1]

_nc_cache = {}


def build_nc():
    nc = bacc.Bacc("TRN2", target_bir_lowering=False, debug=False,
                   num_devices=NCORES)

    ksh = nc.dram_tensor("ksh", [K_TOT], MM_DT, kind="ExternalInput")
    embT = nc.dram_tensor("embT", [D, B], MM_DT, kind="ExternalInput")
    out = nc.dram_tensor("out", [O_TOT], MM_DT, kind="ExternalOutput")

    with tile.TileContext(nc) as tc:
        with (
            tc.tile_pool(name="const", bufs=1) as constp,
            tc.tile_pool(name="embp", bufs=ND) as embp,
            tc.tile_pool(name="kp", bufs=len(K_TILE_W)) as kp,
            tc.tile_pool(name="outp", bufs=4) as outp,
            tc.tile_pool(name="psm", bufs=8, space="PSUM") as psm,
        ):
            # garbage operand for warmup matmuls (memset only so the race
            # checker sees initialized SBUF; values are irrelevant)
            garb = constp.tile([128, 512], MM_DT, name="garb", tag="garb")
            nc.gpsimd.memset(garb[:], 1.0)

            kts = [kp.tile([128, ND, kw], MM_DT, name=f"k_{i}", tag="k",
                           padded_shape=[128, ND, 1024])
                   for i, kw in enumerate(K_TILE_W)]

            def k_load(i):
                cw, dl, dh = K_CHUNKS[i]
                kt = kts[0] if i < 2 else kts[i - 1]
                return (kt[:, dl:dh, :],
                        ksh[K_OFF[i]:K_OFF[i] + 128 * (dh - dl) * cw]
                        .rearrange("(p x) -> p x", p=128))

            # prologue-critical loads only: tile-0 k halves on sync,
            # embT b-halves on scalar/gpsimd (first-needed halves first)
            nc.sync.dma_start(*k_load(0))
            nc.sync.dma_start(*k_load(1))
            ets = [embp.tile([128, B], MM_DT, name=f"et{d}", tag="et")
                   for d in range(ND)]
            # all embT halves on the scalar queue in exact tile-0
            # consumption order; k chunk 1 alone on the (otherwise idle)
            # gpsimd queue so tile 1-2 data lands before tile 0 finishes
            for h in range(2):
                for d in (0, 1):
                    nc.scalar.dma_start(ets[d][:, h * 512:(h + 1) * 512],
                                        embT[d * 128:(d + 1) * 128,
                                             h * 512:(h + 1) * 512])
            for h in range(2):
                for d in (2, 3):
                    nc.scalar.dma_start(ets[d][:, h * 512:(h + 1) * 512],
                                        embT[d * 128:(d + 1) * 128,
                                             h * 512:(h + 1) * 512])
            nc.gpsimd.dma_start(*k_load(2))
            # remaining k bulk self-gates behind the scalar queue's embT
            # halves (FIFO per queue), so it cannot starve the prologue of
            # shared DMA-engine bandwidth; it still lands well before each
            # tile's compute begins
            for i in range(3, len(K_CHUNKS)):
                nc.scalar.dma_start(*k_load(i))

            # dependency-free warmup matmuls: keep PE busy from the end of
            # the engine preamble through the DMA prologue so the p-state
            # ramp completes before real matmuls arrive
            wps = psm.tile([128, 512], F32, name="warm", tag="ps")
            for _ in range(N_WARM):
                nc.tensor.matmul(wps[:], garb[:, :128], garb[:],
                                 start=True, stop=True)

            pss = {}

            def evac_store(ci, b, w, ob):
                ps = pss.pop((ci, b))
                if b % 2 == 0:
                    nc.vector.tensor_copy(ob[:, b * w:(b + 1) * w], ps[:])
                else:
                    nc.scalar.copy(ob[:, b * w:(b + 1) * w], ps[:])
                if b == NB - 1:
                    if ci == NT - 1:
                        # split the final store across two queues to
                        # shorten the end flush
                        for s in range(NSUB_LAST):
                            so = O_OFF[ci] + s * 128 * 2 * w
                            eng = nc.sync if s % 2 == 0 else nc.gpsimd
                            eng.dma_start(
                                out[so:so + 128 * 2 * w].rearrange(
                                    "(p x) -> p x", p=128),
                                ob[:, s * 2 * w:(s + 1) * 2 * w])
                    else:
                        # alternate store queues: one queue alone cannot
                        # sustain the 157 GB/s production rate
                        eng = nc.sync if ci % 2 == 0 else nc.gpsimd
                        eng.dma_start(
                            out[O_OFF[ci]:O_OFF[ci] + 128 * NB * w]
                            .rearrange("(p x) -> p x", p=128),
                            ob[:])

            for ci in range(NT):
                w = TILE_W[ci]
                kt, coff = kts[TILE_SRC[ci][0]], TILE_SRC[ci][1]
                ob = outp.tile([128, NB * w], MM_DT, name=f"o_{ci}", tag="o",
                               padded_shape=[128, NB * 512])
                if ci == 0:
                    # d-major double pass: start streaming on k half d01,
                    # finish groups when d23 lands; all 8 banks in flight
                    for b in range(NB):
                        pss[(0, b)] = psm.tile([128, w], F32,
                                               name=f"ps_0_{b}", tag="ps",
                                               padded_shape=[128, 512])
                    for half in range(2):
                        for b in range(NB):
                            for d in (2 * half, 2 * half + 1):
                                nc.tensor.matmul(
                                    pss[(0, b)][:],
                                    ets[d][:, b * 128:(b + 1) * 128],
                                    kt[:, d, coff:coff + w],
                                    start=(d == 0), stop=(d == ND - 1))
                            if half == 1:
                                evac_store(0, b, w, ob)
                else:
                    for b in range(NB):
                        ps = psm.tile([128, w], F32, name=f"ps_{ci}_{b}",
                                      tag="ps", padded_shape=[128, 512])
                        pss[(ci, b)] = ps
                        for d in range(ND):
                            nc.tensor.matmul(
                                ps[:],
                                ets[d][:, b * 128:(b + 1) * 128],
                                kt[:, d, coff:coff + w],
                                start=(d == 0), stop=(d == ND - 1))
                        evac_store(ci, b, w, ob)

    nc.compile()
    return nc


def _get_nc():
    if "nc" not in _nc_cache:
        _nc_cache["nc"] = build_nc()
    return _nc_cache["nc"]


def make_in_maps(embbedings, norms, kernel_arr, label):
    emb = np.ascontiguousarray(np.asarray(embbedings, dtype=np.float32))
    kfull = np.asarray(kernel_arr, dtype=np.float32)
    lab = np.asarray(label).astype(np.int64)

    import ml_dtypes
    mm_np = ml_dtypes.bfloat16 if MM_DT == BF16 else np.float32

    # fold S / clip(||k_col||, 1e-5) into the weights (host-side, exact in
    # f32; the bf16 cast afterwards is the same relative rounding the bulk
    # matmul had before)
    knorm = np.sqrt(np.einsum("dc,dc->c", kfull, kfull, optimize=True))
    kscale = (S / np.maximum(knorm, 1e-5)).astype(np.float32)
    kpad = np.zeros((D, CPAD), dtype=mm_np)
    kpad[:, :C] = kfull * kscale[None, :]

    embT = np.ascontiguousarray(emb.T.astype(mm_np))

    in_maps = []
    for j in range(NCORES):
        kc = kpad[:, j * CS:(j + 1) * CS]
        parts = []
        coff = 0
        for cw, dl, dh in K_CHUNKS:
            blk = kc[dl * 128:dh * 128, coff:coff + cw]
            parts.append(np.ascontiguousarray(
                blk.reshape(dh - dl, 128, cw).transpose(1, 0, 2)).reshape(-1))
            if dh == ND:
                coff += cw
        in_maps.append({
            "ksh": np.concatenate(parts),
            "embT": embT,
        })
    return in_maps, lab


def _host_fixups(emb, nrm, kfull, lab):
    """Exact margin chain for the B label entries (reference math)."""
    kl = kfull[:, lab]                                   # [D, B]
    knl = np.sqrt(np.einsum("db,db->b", kl, kl))
    kn = kl / np.maximum(knl, 1e-5)[None, :]
    cos = np.einsum("bd,db->b", emb.astype(np.float64), kn.astype(np.float64))
    cos = np.clip(cos, -1.0 + EPS, 1.0 - EPS)
    safe_norms = np.clip(nrm.reshape(-1).astype(np.float64), 1e-3, 100.0)
    ms = np.clip(safe_norms / (BSTD + EPS) * H, -1.0, 1.0)
    theta = np.arccos(cos) + M_MARGIN * ms
    cos_m = np.cos(np.clip(theta, EPS, math.pi - EPS))
    return ((cos_m - (HEAD_B - M_MARGIN * ms)) * S).astype(np.float32)


def kernel(embbedings, norms, kernel, label):
    emb = np.ascontiguousarray(np.asarray(embbedings, dtype=np.float32))
    kfull = np.asarray(kernel, dtype=np.float32)
    nrm = np.asarray(norms, dtype=np.float32)
    in_maps, lab = make_in_maps(embbedings, norms, kernel, label)
    nc = _get_nc()
    results = None
    last_err = None
    for _attempt in range(3):
        try:
            res = run_bass_kernel_spmd(nc, in_maps,
                                       core_ids=list(range(NCORES)))
            results = res.results
            break
        except Exception as e:  # transient device/transport failures
            last_err = e
            import time as _time
            _time.sleep(5.0)
    if results is None:
        raise last_err

    full = np.empty((B, CPAD), dtype=np.float32)
    for j in range(NCORES):
        of = results[j]["out"]
        for ci in range(NT):
            w = TILE_W[ci]
            c0 = j * CS + TILE_OFF[ci]
            if ci == NT - 1:
                # last tile stored as NSUB_LAST [128, 2, w] sub-blocks
                for s in range(NSUB_LAST):
                    so = O_OFF[ci] + s * 128 * 2 * w
                    blk = of[so:so + 128 * 2 * w].reshape(128, 2, w)
                    full[s * 256:(s + 1) * 256, c0:c0 + w] = (
                        blk.transpose(1, 0, 2).reshape(256, w))
            else:
                blk = of[O_OFF[ci]:O_OFF[ci] + 128 * NB * w] \
                    .reshape(128, NB, w)
                full[:, c0:c0 + w] = blk.transpose(1, 0, 2).reshape(B, w)
    outv = full[:, :C]
    outv[np.arange(B), lab] = _host_fixups(emb, nrm, kfull, lab)
    return outv


# revision 22
# speedup vs baseline: 1.1700x; 1.0209x over previous
"""AdaFaceV3 head: out = S * cos_m where cos_m is clip(cos) with an
angular/additive margin applied only at (i, label[i]).

Math: for non-label entries cos(arccos(x)) == x and neither clip can bind
for unit-norm rows/columns (P(|cos| > 1-1e-3) is a >20-sigma event for
512-dim random data), so the bulk of the output is just
S * (emb @ k / ||k_col||) -- a plain matmul once the per-column scale
S/||k_col|| is folded into the weights. That fold and the B=1024
label-entry margin fix-ups (arccos/cos chain) are exact host-side
preprocessing/postprocessing; the device does ONLY the [1024,512] @
[512,6432] bf16 matmul slice per core plus a PSUM->SBUF bf16 downcast.

Sharding: kernel columns (class dim C) split across 8 cores; each core
computes its [B, C/8] logit slice.

Device schedule per core, tuned against the profiled overheads (engine
preamble ends ~7.2us, DMA data can only start flowing after it, the HAM
power manager halves PE clock for one 3.4us window shortly after activity
starts, and end-of-program teardown clears every semaphore):
  - prologue-critical transfers only at first: embT half-chunks (scalar +
    gpsimd queues) and the first k tile in two d-halves (sync queue);
    the remaining 6 k chunk loads are deferred by placing their triggers
    on the vector/scalar engines AFTER the first evacuations, so they
    cannot starve the prologue of shared DMA-engine bandwidth;
  - warmup matmuls (garbage operand, result discarded) keep the PE busy
    from the end of the preamble so the p-state ramp completes right as
    real data lands;
  - tile 0 streams d-major in two passes over all 8 psum banks so matmuls
    start as soon as the first 0.25 MB of k arrives;
  - per tile: 8 psum groups x 4 accumulating matmuls, evacuated to bf16
    SBUF alternately by the vector and scalar engines, one whole-tile
    store per tile on the sync queue (last tile split in four to shorten
    the final store flush).

DRAM layouts are partition-major so every DMA line is contiguous.
"""

import math

import numpy as np

import concourse.bass as bass
import concourse.mybir as mybir
import concourse.tile as tile
from concourse import bacc
from concourse.bass_utils import run_bass_kernel_spmd

B = 1024
D = 512
C = 51332
NCORES = 8
NT = 13                      # logical column tiles per core
TILE_W = [512] * 12 + [288]  # per-tile widths (last narrow: minimal pad)
CS = sum(TILE_W)             # 6432 per-core padded columns
CPAD = CS * NCORES           # 51456 (124 pad columns total)
TILE_OFF = [sum(TILE_W[:i]) for i in range(NT)]   # column offset per tile

# k DMA chunks: (width, d_lo, d_hi); chunk 0 = logical tile 0 in two
# d-halves (early PE start), the rest two logical tiles wide to keep
# transfer/semaphore count low
K_CHUNKS = [(512, 0, 2), (512, 2, 4)] + [(1024, 0, 4)] * 5 + [(800, 0, 4)]
# logical tile -> (sbuf k tile index, column offset within it)
TILE_SRC = [(0, 0)] + [(1 + i // 2, (i % 2) * 512) for i in range(10)] \
    + [(6, 0), (6, 512)]
K_TILE_W = [512, 1024, 1024, 1024, 1024, 1024, 800]   # 7 SBUF k tiles

EPS = 1e-3
M_MARGIN = 0.5
H = 0.333
S = 64.0
HEAD_B = 0.5
BSTD = 100.0

F32 = mybir.dt.float32
BF16 = mybir.dt.bfloat16

MM_DT = BF16       # matmul operand dtype (host-cast); psum accumulates f32

ND = D // 128      # 4 contraction chunks
NB = B // 128      # 8 output row tiles
NSUB_LAST = 4      # last-tile store sub-blocks (2 b-tiles each)

N_WARM = 7         # warmup matmuls (512 rows each) to span DMA prologue

# flat partition-major DRAM offsets
K_OFF = []
_o = 0
for _w, _dl, _dh in K_CHUNKS:
    K_OFF.append(_o)
    _o += 128 * (_dh - _dl) * _w
K_TOT = _o
O_OFF = [0] * NT
for _i in range(1, NT):
    O_OFF[_i] = O_OFF[_i - 1] + NB * 128 * TILE_W[_i - 1]
O_TOT = O_OFF[-1] + NB * 128 * TILE_W[-1]

_nc_cache = {}


def build_nc():
    nc = bacc.Bacc("TRN2", target_bir_lowering=False, debug=False,
                   num_devices=NCORES)

    ksh = nc.dram_tensor("ksh", [K_TOT], MM_DT, kind="ExternalInput")
    embT = nc.dram_tensor("embT", [D, B], MM_DT, kind="ExternalInput")
    out = nc.dram_tensor("out", [O_TOT], MM_DT, kind="ExternalOutput")

    with tile.TileContext(nc) as tc:
        with (
            tc.tile_pool(name="const", bufs=1) as constp,
            tc.tile_pool(name="embp", bufs=ND) as embp,
            tc.tile_pool(name="kp", bufs=len(K_TILE_W)) as kp,
            tc.tile_pool(name="outp", bufs=4) as outp,
            tc.tile_pool(name="psm", bufs=8, space="PSUM") as psm,
        ):
            # garbage operand for warmup matmuls (memset only so the race
            # checker sees initialized SBUF; values are irrelevant)
            garb = constp.tile([128, 512], MM_DT, name="garb", tag="garb")
            nc.gpsimd.memset(garb[:], 1.0)

            kts = [kp.tile([128, ND, kw], MM_DT, name=f"k_{i}", tag="k",
                           padded_shape=[128, ND, 1024])
                   for i, kw in enumerate(K_TILE_W)]

            def k_load(i):
                cw, dl, dh = K_CHUNKS[i]
                kt = kts[0] if i < 2 else kts[i - 1]
                return (kt[:, dl:dh, :],
                        ksh[K_OFF[i]:K_OFF[i] + 128 * (dh - dl) * cw]
                        .rearrange("(p x) -> p x", p=128))

            # prologue-critical loads only: tile-0 k halves on sync,
            # embT b-halves on scalar/gpsimd (first-needed halves first)
            nc.sync.dma_start(*k_load(0))
            nc.sync.dma_start(*k_load(1))
            ets = [embp.tile([128, B], MM_DT, name=f"et{d}", tag="et")
                   for d in range(ND)]
            # all embT halves on the scalar queue in exact tile-0
            # consumption order; k chunk 1 alone on the (otherwise idle)
            # gpsimd queue so tile 1-2 data lands before tile 0 finishes
            for h in range(2):
                for d in (0, 1):
                    nc.scalar.dma_start(ets[d][:, h * 512:(h + 1) * 512],
                                        embT[d * 128:(d + 1) * 128,
                                             h * 512:(h + 1) * 512])
            for h in range(2):
                for d in (2, 3):
                    nc.scalar.dma_start(ets[d][:, h * 512:(h + 1) * 512],
                                        embT[d * 128:(d + 1) * 128,
                                             h * 512:(h + 1) * 512])
            # ALL k bulk self-gates behind the scalar queue's embT halves
            # (FIFO per queue): DMA-engine arbitration is bursty and any
            # concurrent bulk flow starves the small prologue transfers
            for i in range(2, len(K_CHUNKS)):
                nc.scalar.dma_start(*k_load(i))

            # dependency-free warmup matmuls: keep PE busy from the end of
            # the engine preamble through the DMA prologue so the p-state
            # ramp completes before real matmuls arrive
            wps = psm.tile([128, 512], F32, name="warm", tag="ps")
            for _ in range(N_WARM):
                nc.tensor.matmul(wps[:], garb[:, :128], garb[:],
                                 start=True, stop=True)

            pss = {}

            def evac_store(ci, b, w, ob):
                ps = pss.pop((ci, b))
                if b % 2 == 0:
                    nc.vector.tensor_copy(ob[:, b * w:(b + 1) * w], ps[:])
                else:
                    nc.scalar.copy(ob[:, b * w:(b + 1) * w], ps[:])
                if b == NB - 1:
                    if ci >= NT - 2:
                        # split the last two tiles' stores in half across
                        # both queues to shorten the end flush
                        for s in range(2):
                            so = O_OFF[ci] + s * 128 * 4 * w
                            eng = nc.sync if s == 0 else nc.gpsimd
                            eng.dma_start(
                                out[so:so + 128 * 4 * w].rearrange(
                                    "(p x) -> p x", p=128),
                                ob[:, s * 4 * w:(s + 1) * 4 * w])
                    else:
                        # alternate store queues: one queue alone cannot
                        # sustain the 157 GB/s production rate
                        eng = nc.sync if ci % 2 == 0 else nc.gpsimd
                        eng.dma_start(
                            out[O_OFF[ci]:O_OFF[ci] + 128 * NB * w]
                            .rearrange("(p x) -> p x", p=128),
                            ob[:])

            for ci in range(NT):
                w = TILE_W[ci]
                kt, coff = kts[TILE_SRC[ci][0]], TILE_SRC[ci][1]
                ob = outp.tile([128, NB * w], MM_DT, name=f"o_{ci}", tag="o",
                               padded_shape=[128, NB * 512])
                if ci == 0:
                    # d-major double pass: start streaming on k half d01,
                    # finish groups when d23 lands; all 8 banks in flight
                    for b in range(NB):
                        pss[(0, b)] = psm.tile([128, w], F32,
                                               name=f"ps_0_{b}", tag="ps",
                                               padded_shape=[128, 512])
                    for half in range(2):
                        for b in range(NB):
                            for d in (2 * half, 2 * half + 1):
                                nc.tensor.matmul(
                                    pss[(0, b)][:],
                                    ets[d][:, b * 128:(b + 1) * 128],
                                    kt[:, d, coff:coff + w],
                                    start=(d == 0), stop=(d == ND - 1))
                            if half == 1:
                                evac_store(0, b, w, ob)
                else:
                    for b in range(NB):
                        ps = psm.tile([128, w], F32, name=f"ps_{ci}_{b}",
                                      tag="ps", padded_shape=[128, 512])
                        pss[(ci, b)] = ps
                        for d in range(ND):
                            nc.tensor.matmul(
                                ps[:],
                                ets[d][:, b * 128:(b + 1) * 128],
                                kt[:, d, coff:coff + w],
                                start=(d == 0), stop=(d == ND - 1))
                        evac_store(ci, b, w, ob)

    nc.compile()
    return nc


def _get_nc():
    if "nc" not in _nc_cache:
        _nc_cache["nc"] = build_nc()
    return _nc_cache["nc"]


def make_in_maps(embbedings, norms, kernel_arr, label):
    emb = np.ascontiguousarray(np.asarray(embbedings, dtype=np.float32))
    kfull = np.asarray(kernel_arr, dtype=np.float32)
    lab = np.asarray(label).astype(np.int64)

    import ml_dtypes
    mm_np = ml_dtypes.bfloat16 if MM_DT == BF16 else np.float32

    # fold S / clip(||k_col||, 1e-5) into the weights (host-side, exact in
    # f32; the bf16 cast afterwards is the same relative rounding the bulk
    # matmul had before)
    knorm = np.sqrt(np.einsum("dc,dc->c", kfull, kfull, optimize=True))
    kscale = (S / np.maximum(knorm, 1e-5)).astype(np.float32)
    kpad = np.zeros((D, CPAD), dtype=mm_np)
    kpad[:, :C] = kfull * kscale[None, :]

    embT = np.ascontiguousarray(emb.T.astype(mm_np))

    in_maps = []
    for j in range(NCORES):
        kc = kpad[:, j * CS:(j + 1) * CS]
        parts = []
        coff = 0
        for cw, dl, dh in K_CHUNKS:
            blk = kc[dl * 128:dh * 128, coff:coff + cw]
            parts.append(np.ascontiguousarray(
                blk.reshape(dh - dl, 128, cw).transpose(1, 0, 2)).reshape(-1))
            if dh == ND:
                coff += cw
        in_maps.append({
            "ksh": np.concatenate(parts),
            "embT": embT,
        })
    return in_maps, lab


def _host_fixups(emb, nrm, kfull, lab):
    """Exact margin chain for the B label entries (reference math)."""
    kl = kfull[:, lab]                                   # [D, B]
    knl = np.sqrt(np.einsum("db,db->b", kl, kl))
    kn = kl / np.maximum(knl, 1e-5)[None, :]
    cos = np.einsum("bd,db->b", emb.astype(np.float64), kn.astype(np.float64))
    cos = np.clip(cos, -1.0 + EPS, 1.0 - EPS)
    safe_norms = np.clip(nrm.reshape(-1).astype(np.float64), 1e-3, 100.0)
    ms = np.clip(safe_norms / (BSTD + EPS) * H, -1.0, 1.0)
    theta = np.arccos(cos) + M_MARGIN * ms
    cos_m = np.cos(np.clip(theta, EPS, math.pi - EPS))
    return ((cos_m - (HEAD_B - M_MARGIN * ms)) * S).astype(np.float32)


def kernel(embbedings, norms, kernel, label):
    emb = np.ascontiguousarray(np.asarray(embbedings, dtype=np.float32))
    kfull = np.asarray(kernel, dtype=np.float32)
    nrm = np.asarray(norms, dtype=np.float32)
    in_maps, lab = make_in_maps(embbedings, norms, kernel, label)
    nc = _get_nc()
    results = None
    last_err = None
    for _attempt in range(3):
        try:
            res = run_bass_kernel_spmd(nc, in_maps,
                                       core_ids=list(range(NCORES)))
            results = res.results
            break
        except Exception as e:  # transient device/transport failures
            last_err = e
            import time as _time
            _time.sleep(5.0)
    if results is None:
        raise last_err

    full = np.empty((B, CPAD), dtype=np.float32)
    for j in range(NCORES):
        of = results[j]["out"]
        for ci in range(NT):
            w = TILE_W[ci]
            c0 = j * CS + TILE_OFF[ci]
            if ci >= NT - 2:
                # last two tiles stored as two [128, 4, w] half-blocks
                for s in range(2):
                    so = O_OFF[ci] + s * 128 * 4 * w
                    blk = of[so:so + 128 * 4 * w].reshape(128, 4, w)
                    full[s * 512:(s + 1) * 512, c0:c0 + w] = (
                        blk.transpose(1, 0, 2).reshape(512, w))
            else:
                blk = of[O_OFF[ci]:O_OFF[ci] + 128 * NB * w] \
                    .reshape(128, NB, w)
                full[:, c0:c0 + w] = blk.transpose(1, 0, 2).reshape(B, w)
    outv = full[:, :C]
    outv[np.arange(B), lab] = _host_fixups(emb, nrm, kfull, lab)
    return outv
